# revision 25
# baseline (speedup 1.0000x reference)
"""Trainium2 Bass kernel for nn_Encoder (embedding_lookup).

Strategy (8-core data-parallel over the entity axis):
  - Host packs weight-derived tables once per call:
      * fused gather table Tg[1536,256] (bf16): species/ability/item feature
        tables folded through their agg_w blocks + their embedding tables,
        plus actions_emb. One row-gather per (entity, feature) then covers
        both the concat@agg_w contribution and emb_sum.
      * one-hot weight block Wp[512,256]: agg_w rows for scalar/boost/bit
        one-hot features (+ hp ratio row /31, agg_b row, -1e9 mask row).
  - Device (per 512-entity tile, transposed layout: features on partitions,
    entities on the free dim):
      * dma_gather (transpose mode) pulls 7*512 rows from Tg in HBM.
      * a selector matmul broadcasts raw feature values across partitions;
        DVE tensor_scalar ops (is_equal / mod+is_ge) turn them into the
        multi-hot matrix; PE matmuls against Wp accumulate into PSUM
        together with the summed gather planes (identity matmul).
      * relu on ACT, then the 256x256 MLP with stationary bf16 weights,
        masked bias via a rank-1 matmul against the (sp>=2) indicator row.
  - Output is written transposed [256, e_core]; the host transposes back.
"""

import sys

sys.path.insert(0, "/opt/trn_rl_repo")

import functools
from contextlib import ExitStack

import numpy as np
import ml_dtypes

import concourse.bass as bass
import concourse.bacc as bacc
import concourse.tile as tile
from concourse import mybir
from concourse.bass_utils import run_bass_kernel_spmd

BF16 = ml_dtypes.bfloat16

# ---------------------------------------------------------------- constants
E = 65536
N_CORES = 8
E_CORE = E // N_CORES
TILE_E = 512

NUM_SPECIES, NUM_ABILITIES, NUM_ITEMS, NUM_ACTIONS = 512, 128, 256, 512
SPECIES, ABILITY, ITEM = 0, 1, 2
SCALAR_FEATS = list(range(3, 16))
SCALAR_MAX = [101, 2, 2, 32, 3, 8, 16, 2, 2, 2, 8, 4, 2]
BOOST_FEATS = list(range(16, 23))
BOOST_MAX = 13
VOL0, VOL8 = 23, 31
TC0, TC1 = 32, 33
MOVE0 = 34
NUM_FEATS = 38
HP_RATIO = 6

SC_TOTAL = sum(SCALAR_MAX)          # 184
SC_OFF = np.concatenate([[0], np.cumsum(SCALAR_MAX)]).astype(int)  # len 14
BOOST_TOTAL = 7 * BOOST_MAX         # 91
N_WORDS = 11                        # 9 volatile + 2 typechange
BITS_TOTAL = 16 * N_WORDS           # 176

# agg_w row offsets of each concat section
AW_SP = 0
AW_AB = 512
AW_IT = 640
AW_SC = 896
AW_BOOST = AW_SC + SC_TOTAL         # 1080
AW_BITS = AW_BOOST + BOOST_TOTAL    # 1171
AW_HP = AW_BITS + BITS_TOTAL        # 1347
CONCAT_DIM = AW_HP + 1              # 1348

# featT (entityT) rows, fp16. values <= 511 so fp16 exact.
FT_SP, FT_AB, FT_IT = 0, 1, 2
FT_SC0 = 3                  # feats 3..15 at rows 3..15
FT_BOOST0 = 16              # feats 16..22 at rows 16..22
FT_BYTE0 = 23               # word wi: lo byte at 23+2wi, hi at 24+2wi
FT_MOVE0 = 45               # rows 45..48
FT_CONST1 = 63              # constant 1.0 row
FT_ROWS = 64

# multi-hot / Wp rows (512 = 4 chunks of 128). Engine ops may only start at
# partitions 0/32/64/96, so the three op kinds (ge/eq/bit) occupy 32-aligned
# row ranges; unused rows inside a range are degenerate (never-true consts).
MH_MASK = 0                 # is_ge:  sp >= 2, consumed as mlp-bias rhs
MH_NULLPAD = 1              # is_ge: -sp >= -1 (selector coef -1) -> Wp -1e9
MH_SC0 = 32                 # 184 scalar one-hot rows -> 32..215 (eq)
MH_BOOST0 = 216             # 91 boost rows -> 216..306 (eq)
MH_BITS0 = 320              # 176 bit rows -> 320..495 (word-major, bit-minor)
MH_ROWS = 512
# hp-ratio (agg_w[1347]*v/31) is folded into feature-6's one-hot block.

# combined gather table rows
TG_SP = 0
TG_AB = 512
TG_IT = 640
TG_MOVE = 896
TG_ROWS = 1536
G_BASES = [TG_SP, TG_AB, TG_IT, TG_MOVE, TG_MOVE, TG_MOVE, TG_MOVE]
G_FT = [FT_SP, FT_AB, FT_IT, FT_MOVE0, FT_MOVE0 + 1, FT_MOVE0 + 2, FT_MOVE0 + 3]
G = 7

MASK_NEG = -1.0e9

# per-chunk op segments: (chunk, lo, hi, kind); all starts 32-aligned
MH_OPS = [
    (0, 0, 32, "ge"),      # mask row, nullpad row, degenerate rest
    (0, 32, 64, "eq"),     # [32,64) start allows only 32 partitions
    (0, 64, 128, "eq"),
    (1, 0, 128, "eq"),
    (2, 0, 64, "eq"),
    (2, 64, 128, "bit"),
    (3, 0, 128, "bit"),
]


def _mh_row_meta(bit_cvt_bias):
    """Per mh-row: selector coef (signed) and compare consts.

    Bit rows use a fractional selector coef 2^-jj: the on-device f32->i16
    convert then yields (v >> jj), AND 1 and is_gt 0 give the bit.
    bit_cvt_bias compensates the convert's rounding mode: hardware rounds
    to nearest-even, so 2^-9 - 0.5 keeps RN(q + frac + bias) == q for all
    frac in [0, 1). CoreSim truncates (bias 0.0).
    """
    coef = np.zeros((FT_ROWS, MH_ROWS), np.float32)    # selector matrix
    ceq = np.full(MH_ROWS, 999.0, dtype=np.float32)    # eq/ge compare const
    coef[FT_SP, MH_MASK] = 1.0
    ceq[MH_MASK] = 2.0                                  # is_ge 2
    coef[FT_SP, MH_NULLPAD] = -1.0
    ceq[MH_NULLPAD] = -1.0                              # -sp >= -1
    for i in range(13):
        for v in range(SCALAR_MAX[i]):
            r = MH_SC0 + SC_OFF[i] + v
            coef[FT_SC0 + i, r] = 1.0
            ceq[r] = float(v)
    for b in range(7):
        for v in range(BOOST_MAX):
            r = MH_BOOST0 + 13 * b + v
            coef[FT_BOOST0 + b, r] = 1.0
            ceq[r] = float(v)
    for wi in range(N_WORDS):
        for j in range(16):
            r = MH_BITS0 + 16 * wi + j
            jj = j % 8
            coef[FT_BYTE0 + 2 * wi + (1 if j >= 8 else 0), r] = 2.0 ** -jj
            coef[FT_CONST1, r] = bit_cvt_bias
    return coef, ceq


BIT_CVT_BIAS = 2.0 ** -9 - 0.5   # HW f32->int rounds to nearest-even
MH_CEQ = _mh_row_meta(0.0)[1]


# ---------------------------------------------------------------- host pack
def _pack_weights(inp):
    """Returns dict of host-packed weight arrays shared by all cores."""
    f32 = np.float32
    agg_w = np.asarray(inp["agg_w"], f32)
    agg_b = np.asarray(inp["agg_b"], f32)
    mlp_w = np.asarray(inp["mlp_w"], f32)
    mlp_b = np.asarray(inp["mlp_b"], f32)

    # fused gather table
    tg = np.zeros((TG_ROWS, 256), f32)
    tg[TG_SP:TG_SP + 512] = (
        np.asarray(inp["species_tbl"], f32) @ agg_w[AW_SP:AW_SP + 512]
        + np.asarray(inp["species_emb"], f32))
    tg[TG_AB:TG_AB + 128] = (
        np.asarray(inp["ability_tbl"], f32) @ agg_w[AW_AB:AW_AB + 128]
        + np.asarray(inp["ability_emb"], f32))
    tg[TG_IT:TG_IT + 256] = (
        np.asarray(inp["item_tbl"], f32) @ agg_w[AW_IT:AW_IT + 256]
        + np.asarray(inp["item_emb"], f32))
    tg[TG_MOVE:TG_MOVE + 512] = np.asarray(inp["actions_emb"], f32)

    # one-hot weight rows
    wp = np.zeros((MH_ROWS, 256), f32)
    wp[MH_SC0:MH_SC0 + SC_TOTAL] = agg_w[AW_SC:AW_SC + SC_TOTAL]
    # hp-ratio fold: feature 6 (scalar idx 3, max 32) one-hot row v also
    # carries (v/31) * agg_w[hp]
    hp_lo = MH_SC0 + SC_OFF[3]
    for v in range(SCALAR_MAX[3]):
        wp[hp_lo + v] += (v / 31.0) * agg_w[AW_HP]
    wp[MH_BOOST0:MH_BOOST0 + BOOST_TOTAL] = agg_w[AW_BOOST:AW_BOOST + BOOST_TOTAL]
    wp[MH_BITS0:MH_BITS0 + BITS_TOTAL] = agg_w[AW_BITS:AW_BITS + BITS_TOTAL]
    wp[MH_NULLPAD] = MASK_NEG

    # [p, (c*2+h)*128 + m] = wp[128c+p, 128h+m]
    wp_h = np.zeros((128, 1024), f32)
    for c in range(4):
        for h in range(2):
            wp_h[:, (c * 2 + h) * 128:(c * 2 + h + 1) * 128] = \
                wp[128 * c:128 * (c + 1), 128 * h:128 * (h + 1)]

    mlpw_h = np.zeros((128, 512), f32)
    for k in range(2):
        for h in range(2):
            mlpw_h[:, (k * 2 + h) * 128:(k * 2 + h + 1) * 128] = \
                mlp_w[128 * k:128 * (k + 1), 128 * h:128 * (h + 1)]

    aggb_h = np.stack([agg_b[:128], agg_b[128:]], axis=1)  # [128, 2]

    # selector B [64, 512] fp16
    b_h = _mh_row_meta(BIT_CVT_BIAS)[0].astype(np.float16)

    cmp_h = MH_CEQ.reshape(4, 128).T.astype(np.float32).copy()   # [128, 4]

    return {
        "tg": np.ascontiguousarray(tg.astype(BF16)),
        "wp": np.ascontiguousarray(wp_h.astype(BF16)),
        "mlpw": np.ascontiguousarray(mlpw_h.astype(BF16)),
        "mlpb": np.ascontiguousarray(mlp_b.astype(BF16).reshape(1, 256)),
        "aggb": np.ascontiguousarray(aggb_h),
        "cmpc": cmp_h,
        "bsel": np.ascontiguousarray(b_h),
        "ident": np.eye(128, dtype=np.float32).astype(BF16),
        "gbase": np.ascontiguousarray(
            np.repeat(np.asarray(G_BASES, np.int16)[None, :, None], 32, axis=2)
            .reshape(1, G * 32).repeat(128, axis=0)),  # [128, 224]
    }


def _pack_entity(ent):
    """Per-core entity-derived arrays: entT fp16 [64, E_CORE], gidx int16."""
    e_core = ent.shape[0]
    ntiles = e_core // TILE_E
    f = np.zeros((e_core, FT_ROWS), np.float16)
    f[:, FT_SP] = ent[:, SPECIES]
    f[:, FT_AB] = ent[:, ABILITY]
    f[:, FT_IT] = ent[:, ITEM]
    for i, feat in enumerate(SCALAR_FEATS):
        f[:, FT_SC0 + i] = ent[:, feat]
    for b, feat in enumerate(BOOST_FEATS):
        f[:, FT_BOOST0 + b] = ent[:, feat]
    words = ent[:, VOL0:TC1 + 1]            # 11 words
    for wi in range(N_WORDS):
        f[:, FT_BYTE0 + 2 * wi] = words[:, wi] & 0xFF
        f[:, FT_BYTE0 + 2 * wi + 1] = words[:, wi] >> 8
    for m in range(4):
        f[:, FT_MOVE0 + m] = ent[:, MOVE0 + m]
    f[:, FT_CONST1] = 1.0
    ent_t = np.ascontiguousarray(f.T)       # [64, e_core]

    v = ent[:, [SPECIES, ABILITY, ITEM, MOVE0, MOVE0 + 1, MOVE0 + 2, MOVE0 + 3]]
    v = v.astype(np.int16).reshape(ntiles, 32, 16, G)    # [t, s, p, g]
    gidx16 = v.transpose(2, 0, 3, 1).reshape(16, ntiles * G * 32)
    # dma_gather ucode: each of the 8 Q7 cores reads its own 16-partition
    # group, so the index block is replicated 8x along partitions.
    gidx = np.ascontiguousarray(np.tile(gidx16, (8, 1)))
    return ent_t, gidx


# ---------------------------------------------------------------- bass build
@functools.lru_cache(maxsize=4)
def _build(e_core):
    ntiles = e_core // TILE_E
    dt = mybir.dt
    nc = bacc.Bacc("TRN2", target_bir_lowering=False, debug=False)

    d_entT = nc.dram_tensor("entT", [FT_ROWS, e_core], dt.float16, kind="ExternalInput").ap()
    d_gidx = nc.dram_tensor("gidx", [128, ntiles * G * 32], dt.int16, kind="ExternalInput").ap()
    d_tg = nc.dram_tensor("tg", [TG_ROWS, 256], dt.bfloat16, kind="ExternalInput").ap()
    d_wp = nc.dram_tensor("wp", [128, 1024], dt.bfloat16, kind="ExternalInput").ap()
    d_mlpw = nc.dram_tensor("mlpw", [128, 512], dt.bfloat16, kind="ExternalInput").ap()
    d_mlpb = nc.dram_tensor("mlpb", [1, 256], dt.bfloat16, kind="ExternalInput").ap()
    d_aggb = nc.dram_tensor("aggb", [128, 2], dt.float32, kind="ExternalInput").ap()
    d_cmpc = nc.dram_tensor("cmpc", [128, 4], dt.float32, kind="ExternalInput").ap()
    d_bsel = nc.dram_tensor("bsel", [FT_ROWS, MH_ROWS], dt.float16, kind="ExternalInput").ap()
    d_ident = nc.dram_tensor("ident", [128, 128], dt.bfloat16, kind="ExternalInput").ap()
    d_gbase = nc.dram_tensor("gbase", [128, G * 32], dt.int16, kind="ExternalInput").ap()
    d_outT = nc.dram_tensor("outT", [256, e_core], dt.float32, kind="ExternalOutput").ap()

    with tile.TileContext(nc) as tc, ExitStack() as ctx:
        cpool = ctx.enter_context(tc.tile_pool(name="consts", bufs=1))
        wpool = ctx.enter_context(tc.tile_pool(name="work", bufs=2))
        gpool = ctx.enter_context(tc.tile_pool(name="gather", bufs=2))
        ppool = ctx.enter_context(tc.tile_pool(name="psum", bufs=1, space="PSUM"))

        # ---- persistent constants
        entT = cpool.tile([FT_ROWS, e_core], dt.float16, tag="entT")
        nc.sync.dma_start(entT[:], d_entT)
        gidx = cpool.tile([128, ntiles * G * 32], dt.int16, tag="gidx")
        nc.sync.dma_start(gidx[:], d_gidx)
        wp = cpool.tile([128, 1024], dt.bfloat16, tag="wp")
        nc.sync.dma_start(wp[:], d_wp)
        mlpw = cpool.tile([128, 512], dt.bfloat16, tag="mlpw")
        nc.sync.dma_start(mlpw[:], d_mlpw)
        mlpb = cpool.tile([1, 256], dt.bfloat16, tag="mlpb")
        nc.sync.dma_start(mlpb[:], d_mlpb)
        aggb = cpool.tile([128, 2], dt.float32, tag="aggb")
        nc.sync.dma_start(aggb[:], d_aggb)
        cmpc = cpool.tile([128, 4], dt.float32, tag="cmpc")
        nc.sync.dma_start(cmpc[:], d_cmpc)
        bsel = cpool.tile([FT_ROWS, MH_ROWS], dt.float16, tag="bsel")
        nc.sync.dma_start(bsel[:], d_bsel)
        ident = cpool.tile([128, 128], dt.bfloat16, tag="ident")
        nc.sync.dma_start(ident[:], d_ident)
        gbase = cpool.tile([128, G * 32], dt.int16, tag="gbase")
        nc.sync.dma_start(gbase[:], d_gbase)

        # persistent gather-index buffer (indices replicated per 16-row group)
        idxb = cpool.tile([128, ntiles * G * 32], dt.int16, tag="idxb")

        for t in range(ntiles):
            es = slice(t * TILE_E, (t + 1) * TILE_E)
            isl = slice(t * G * 32, (t + 1) * G * 32)

            # gather indices for this tile
            nc.vector.tensor_tensor(
                idxb[:, isl], gidx[:, isl], gbase[:], mybir.AluOpType.add)

            # 7*TILE_E row gather from Tg (HBM), transposed output
            gpl = gpool.tile([128, 2 * G * TILE_E], dt.bfloat16, tag="gpl")
            gpl3 = gpl[:].rearrange("p (c j) -> p c j", c=2)
            nc.gpsimd.dma_gather(
                out_ap=gpl3,
                in_ap=d_tg,
                idxs_ap=idxb[:, isl],
                num_idxs=G * TILE_E,
                num_idxs_reg=G * TILE_E,
                elem_size=256,
                transpose=True,
                single_packet=False,
            )

            # selector matmuls: raw[c] = B_c.T @ featT
            raws = []
            for c in range(4):
                raw = ppool.tile([128, TILE_E], dt.float32, tag=f"raw{c}")
                nc.tensor.matmul(
                    raw[:], bsel[:, c * 128:(c + 1) * 128], entT[:, es],
                    start=True, stop=True)
                raws.append(raw)

            # multi-hot construction
            mh = wpool.tile([128, 4 * TILE_E], dt.bfloat16, tag="mh")
            cvti = wpool.tile([128, TILE_E], dt.int16, tag="cvti")
            cvt2 = wpool.tile([128, TILE_E], dt.int16, tag="cvt2")
            rawh = wpool.tile([128, TILE_E], dt.float32, tag="rawh")
            for (c, lo, hi, kind) in MH_OPS:
                dst = mh[lo:hi, c * TILE_E:(c + 1) * TILE_E]
                src = raws[c][lo:hi, :]
                if kind == "eq":
                    nc.vector.tensor_scalar(
                        dst, src, cmpc[lo:hi, c:c + 1], None,
                        mybir.AluOpType.is_equal)
                elif kind == "bit":
                    # raw = v*2^-jj + bias; bit = (v>>jj) - 2*(v>>(jj+1)),
                    # integer shifts realized as RNE-safe f32->i16 casts
                    # (int16 bitwise ops are ~8x slower than casts on DVE).
                    nc.vector.tensor_scalar(
                        rawh[lo:hi, :], src, 0.5, BIT_CVT_BIAS * 0.5,
                        mybir.AluOpType.mult, mybir.AluOpType.add)
                    nc.vector.tensor_copy(cvti[lo:hi, :], src)
                    nc.vector.tensor_copy(cvt2[lo:hi, :], rawh[lo:hi, :])
                    nc.vector.scalar_tensor_tensor(
                        dst, cvt2[lo:hi, :], -2.0, cvti[lo:hi, :],
                        mybir.AluOpType.mult, mybir.AluOpType.add)
                elif kind == "ge":
                    nc.vector.tensor_scalar(
                        dst, src, cmpc[lo:hi, c:c + 1], None,
                        mybir.AluOpType.is_ge)

            # gather-plane sum (+ agg_b on the final combine)
            def plane(g):
                return gpl3[:, :, g * TILE_E:(g + 1) * TILE_E]

            a0 = wpool.tile([128, 2 * TILE_E], dt.bfloat16, tag="a0")
            a03 = a0[:].rearrange("p (c j) -> p c j", c=2)
            nc.vector.tensor_tensor(a03, plane(0), plane(1), mybir.AluOpType.add)
            a1 = wpool.tile([128, 2 * TILE_E], dt.bfloat16, tag="a1")
            a13 = a1[:].rearrange("p (c j) -> p c j", c=2)
            nc.vector.tensor_tensor(a13, plane(2), plane(3), mybir.AluOpType.add)
            a2 = wpool.tile([128, 2 * TILE_E], dt.bfloat16, tag="a2")
            a23 = a2[:].rearrange("p (c j) -> p c j", c=2)
            nc.vector.tensor_tensor(a23, plane(4), plane(5), mybir.AluOpType.add)
            nc.vector.tensor_tensor(a03, a03, a13, mybir.AluOpType.add)
            nc.vector.tensor_tensor(a23, a23, plane(6), mybir.AluOpType.add)
            gs = wpool.tile([128, 2 * TILE_E], dt.bfloat16, tag="gs")
            for h in range(2):
                nc.vector.scalar_tensor_tensor(
                    gs[:, h * TILE_E:(h + 1) * TILE_E],
                    a0[:, h * TILE_E:(h + 1) * TILE_E],
                    aggb[:, h:h + 1],
                    a2[:, h * TILE_E:(h + 1) * TILE_E],
                    mybir.AluOpType.add, mybir.AluOpType.add)

            # x1 = gathers + one-hot part (PSUM accumulation)
            x1 = []
            for h in range(2):
                p = ppool.tile([128, TILE_E], dt.float32, tag=f"x1_{h}")
                nc.tensor.matmul(
                    p[:], ident[:], gs[:, h * TILE_E:(h + 1) * TILE_E],
                    start=True, stop=False)
                for c in range(4):
                    nc.tensor.matmul(
                        p[:], wp[:, (c * 2 + h) * 128:(c * 2 + h + 1) * 128],
                        mh[:, c * TILE_E:(c + 1) * TILE_E],
                        start=False, stop=(c == 3))
                x1.append(p)

            # relu -> xr (bf16)
            xr = wpool.tile([128, 2 * TILE_E], dt.bfloat16, tag="xr")
            for h in range(2):
                nc.scalar.activation(
                    xr[:, h * TILE_E:(h + 1) * TILE_E], x1[h][:],
                    mybir.ActivationFunctionType.Relu)

            # out = xr @ mlp_w + mask*mlp_b
            mrow = mh[MH_MASK:MH_MASK + 1, 0:TILE_E]    # (sp>=2) row, chunk 0
            for h in range(2):
                po = ppool.tile([128, TILE_E], dt.float32, tag=f"out_{h}")
                for k in range(2):
                    nc.tensor.matmul(
                        po[:], mlpw[:, (k * 2 + h) * 128:(k * 2 + h + 1) * 128],
                        xr[:, k * TILE_E:(k + 1) * TILE_E],
                        start=(k == 0), stop=False)
                nc.tensor.matmul(
                    po[:], mlpb[:, h * 128:(h + 1) * 128], mrow,
                    start=False, stop=True)
                ob = wpool.tile([128, TILE_E], dt.float32, tag=f"ob{h}")
                nc.scalar.activation(
                    ob[:], po[:], mybir.ActivationFunctionType.Copy)
                nc.sync.dma_start(d_outT[h * 128:(h + 1) * 128, es], ob[:])

    nc.compile()
    return nc


# ---------------------------------------------------------------- entry
def _make_in_maps(inputs, n_cores, e_core):
    ent = np.asarray(inputs["entity"], np.int32)
    w = _pack_weights(inputs)
    in_maps = []
    for i in range(n_cores):
        ent_t, gidx = _pack_entity(ent[i * e_core:(i + 1) * e_core])
        in_maps.append({
            "entT": ent_t, "gidx": gidx, "tg": w["tg"], "wp": w["wp"],
            "mlpw": w["mlpw"], "mlpb": w["mlpb"], "aggb": w["aggb"],
            "cmpc": w["cmpc"],
            "bsel": w["bsel"], "ident": w["ident"], "gbase": w["gbase"],
        })
    return in_maps


def kernel(**inputs):
    nc = _build(E_CORE)
    in_maps = _make_in_maps(inputs, N_CORES, E_CORE)
    res = run_bass_kernel_spmd(nc, in_maps, list(range(N_CORES)))
    out = np.concatenate(
        [np.ascontiguousarray(res.results[i]["outT"].T) for i in range(N_CORES)],
        axis=0)
    return out


def run_traced(inputs):
    """test.py helper: returns (output, exec_time_ns)."""
    nc = _build(E_CORE)
    in_maps = _make_in_maps(inputs, N_CORES, E_CORE)
    # warmup: connects the axon client (profile hook needs it) + NEFF cache
    run_bass_kernel_spmd(nc, in_maps, list(range(N_CORES)))
    res = run_bass_kernel_spmd(nc, in_maps, list(range(N_CORES)), trace=True)
    out = np.concatenate(
        [np.ascontiguousarray(res.results[i]["outT"].T) for i in range(N_CORES)],
        axis=0)
    return out, res.exec_time_ns


# revision 26
# speedup vs baseline: 1.0868x; 1.0868x over previous
"""Trainium2 Bass kernel for nn_Encoder (embedding_lookup).

Strategy (8-core data-parallel over the entity axis):
  - Host packs weight-derived tables once per call:
      * fused gather table Tg[1536,256] (bf16): species/ability/item feature
        tables folded through their agg_w blocks + their embedding tables,
        plus actions_emb. One row-gather per (entity, feature) then covers
        both the concat@agg_w contribution and emb_sum.
      * one-hot weight block Wp[512,256]: agg_w rows for scalar/boost/bit
        one-hot features (+ hp ratio row /31, agg_b row, -1e9 mask row).
  - Device (per 512-entity tile, transposed layout: features on partitions,
    entities on the free dim):
      * dma_gather (transpose mode) pulls 7*512 rows from Tg in HBM.
      * a selector matmul broadcasts raw feature values across partitions;
        DVE tensor_scalar ops (is_equal / mod+is_ge) turn them into the
        multi-hot matrix; PE matmuls against Wp accumulate into PSUM
        together with the summed gather planes (identity matmul).
      * relu on ACT, then the 256x256 MLP with stationary bf16 weights,
        masked bias via a rank-1 matmul against the (sp>=2) indicator row.
  - Output is written transposed [256, e_core]; the host transposes back.
"""

import sys

sys.path.insert(0, "/opt/trn_rl_repo")

import functools
from contextlib import ExitStack

import numpy as np
import ml_dtypes

import concourse.bass as bass
import concourse.bacc as bacc
import concourse.tile as tile
from concourse import mybir
from concourse.bass_utils import run_bass_kernel_spmd

BF16 = ml_dtypes.bfloat16

# ---------------------------------------------------------------- constants
E = 65536
N_CORES = 8
E_CORE = E // N_CORES
TILE_E = 512

NUM_SPECIES, NUM_ABILITIES, NUM_ITEMS, NUM_ACTIONS = 512, 128, 256, 512
SPECIES, ABILITY, ITEM = 0, 1, 2
SCALAR_FEATS = list(range(3, 16))
SCALAR_MAX = [101, 2, 2, 32, 3, 8, 16, 2, 2, 2, 8, 4, 2]
BOOST_FEATS = list(range(16, 23))
BOOST_MAX = 13
VOL0, VOL8 = 23, 31
TC0, TC1 = 32, 33
MOVE0 = 34
NUM_FEATS = 38
HP_RATIO = 6

SC_TOTAL = sum(SCALAR_MAX)          # 184
SC_OFF = np.concatenate([[0], np.cumsum(SCALAR_MAX)]).astype(int)  # len 14
BOOST_TOTAL = 7 * BOOST_MAX         # 91
N_WORDS = 11                        # 9 volatile + 2 typechange
BITS_TOTAL = 16 * N_WORDS           # 176

# agg_w row offsets of each concat section
AW_SP = 0
AW_AB = 512
AW_IT = 640
AW_SC = 896
AW_BOOST = AW_SC + SC_TOTAL         # 1080
AW_BITS = AW_BOOST + BOOST_TOTAL    # 1171
AW_HP = AW_BITS + BITS_TOTAL        # 1347
CONCAT_DIM = AW_HP + 1              # 1348

# featT (entityT) rows, fp16. values <= 511 so fp16 exact.
FT_SP, FT_AB, FT_IT = 0, 1, 2
FT_SC0 = 3                  # feats 3..15 at rows 3..15
FT_BOOST0 = 16              # feats 16..22 at rows 16..22
FT_BYTE0 = 23               # word wi: lo byte at 23+2wi, hi at 24+2wi
FT_MOVE0 = 45               # rows 45..48
FT_CONST1 = 63              # constant 1.0 row
FT_ROWS = 64

# multi-hot / Wp rows (512 = 4 chunks of 128). Engine ops may only start at
# partitions 0/32/64/96, so the three op kinds (ge/eq/bit) occupy 32-aligned
# row ranges; unused rows inside a range are degenerate (never-true consts).
MH_MASK = 0                 # is_ge:  sp >= 2, consumed as mlp-bias rhs
MH_NULLPAD = 1              # is_ge: -sp >= -1 (selector coef -1) -> Wp -1e9
MH_SC0 = 32                 # 184 scalar one-hot rows -> 32..215 (eq)
MH_BOOST0 = 216             # 91 boost rows -> 216..306 (eq)
MH_BITS0 = 320              # 176 bit rows -> 320..495 (word-major, bit-minor)
MH_ROWS = 512
# hp-ratio (agg_w[1347]*v/31) is folded into feature-6's one-hot block.

# combined gather table rows
TG_SP = 0
TG_AB = 512
TG_IT = 640
TG_MOVE = 896
TG_ROWS = 1536
G_BASES = [TG_SP, TG_AB, TG_IT, TG_MOVE, TG_MOVE, TG_MOVE, TG_MOVE]
G_FT = [FT_SP, FT_AB, FT_IT, FT_MOVE0, FT_MOVE0 + 1, FT_MOVE0 + 2, FT_MOVE0 + 3]
G = 7

MASK_NEG = -1.0e9

# per-chunk op segments: (chunk, lo, hi, kind); all starts 32-aligned
MH_OPS = [
    (0, 0, 32, "ge"),      # mask row, nullpad row, degenerate rest
    (0, 32, 64, "eq"),     # [32,64) start allows only 32 partitions
    (0, 64, 128, "eq"),
    (1, 0, 128, "eq"),
    (2, 0, 64, "eq"),
    (2, 64, 128, "bit"),
    (3, 0, 128, "bit"),
]


def _mh_row_meta(bit_cvt_bias):
    """Per mh-row: selector coef (signed) and compare consts.

    Bit rows use a fractional selector coef 2^-jj: the on-device f32->i16
    convert then yields (v >> jj), AND 1 and is_gt 0 give the bit.
    bit_cvt_bias compensates the convert's rounding mode: hardware rounds
    to nearest-even, so 2^-9 - 0.5 keeps RN(q + frac + bias) == q for all
    frac in [0, 1). CoreSim truncates (bias 0.0).
    """
    coef = np.zeros((FT_ROWS, MH_ROWS), np.float32)    # selector matrix
    ceq = np.full(MH_ROWS, 999.0, dtype=np.float32)    # eq/ge compare const
    coef[FT_SP, MH_MASK] = 1.0
    ceq[MH_MASK] = 2.0                                  # is_ge 2
    coef[FT_SP, MH_NULLPAD] = -1.0
    ceq[MH_NULLPAD] = -1.0                              # -sp >= -1
    for i in range(13):
        for v in range(SCALAR_MAX[i]):
            r = MH_SC0 + SC_OFF[i] + v
            coef[FT_SC0 + i, r] = 1.0
            ceq[r] = float(v)
    for b in range(7):
        for v in range(BOOST_MAX):
            r = MH_BOOST0 + 13 * b + v
            coef[FT_BOOST0 + b, r] = 1.0
            ceq[r] = float(v)
    for wi in range(N_WORDS):
        for j in range(16):
            r = MH_BITS0 + 16 * wi + j
            jj = j % 8
            coef[FT_BYTE0 + 2 * wi + (1 if j >= 8 else 0), r] = 2.0 ** -jj
            coef[FT_CONST1, r] = bit_cvt_bias
    return coef, ceq


BIT_CVT_BIAS = 2.0 ** -9 - 0.5   # HW f32->int rounds to nearest-even
MH_CEQ = _mh_row_meta(0.0)[1]


# ---------------------------------------------------------------- host pack
def _pack_weights(inp):
    """Returns dict of host-packed weight arrays shared by all cores."""
    f32 = np.float32
    agg_w = np.asarray(inp["agg_w"], f32)
    agg_b = np.asarray(inp["agg_b"], f32)
    mlp_w = np.asarray(inp["mlp_w"], f32)
    mlp_b = np.asarray(inp["mlp_b"], f32)

    # fused gather table
    tg = np.zeros((TG_ROWS, 256), f32)
    tg[TG_SP:TG_SP + 512] = (
        np.asarray(inp["species_tbl"], f32) @ agg_w[AW_SP:AW_SP + 512]
        + np.asarray(inp["species_emb"], f32))
    tg[TG_AB:TG_AB + 128] = (
        np.asarray(inp["ability_tbl"], f32) @ agg_w[AW_AB:AW_AB + 128]
        + np.asarray(inp["ability_emb"], f32))
    tg[TG_IT:TG_IT + 256] = (
        np.asarray(inp["item_tbl"], f32) @ agg_w[AW_IT:AW_IT + 256]
        + np.asarray(inp["item_emb"], f32))
    tg[TG_MOVE:TG_MOVE + 512] = np.asarray(inp["actions_emb"], f32)

    # one-hot weight rows
    wp = np.zeros((MH_ROWS, 256), f32)
    wp[MH_SC0:MH_SC0 + SC_TOTAL] = agg_w[AW_SC:AW_SC + SC_TOTAL]
    # hp-ratio fold: feature 6 (scalar idx 3, max 32) one-hot row v also
    # carries (v/31) * agg_w[hp]
    hp_lo = MH_SC0 + SC_OFF[3]
    for v in range(SCALAR_MAX[3]):
        wp[hp_lo + v] += (v / 31.0) * agg_w[AW_HP]
    wp[MH_BOOST0:MH_BOOST0 + BOOST_TOTAL] = agg_w[AW_BOOST:AW_BOOST + BOOST_TOTAL]
    wp[MH_BITS0:MH_BITS0 + BITS_TOTAL] = agg_w[AW_BITS:AW_BITS + BITS_TOTAL]
    wp[MH_NULLPAD] = MASK_NEG

    # [p, (c*2+h)*128 + m] = wp[128c+p, 128h+m]
    wp_h = np.zeros((128, 1024), f32)
    for c in range(4):
        for h in range(2):
            wp_h[:, (c * 2 + h) * 128:(c * 2 + h + 1) * 128] = \
                wp[128 * c:128 * (c + 1), 128 * h:128 * (h + 1)]

    mlpw_h = np.zeros((128, 512), f32)
    for k in range(2):
        for h in range(2):
            mlpw_h[:, (k * 2 + h) * 128:(k * 2 + h + 1) * 128] = \
                mlp_w[128 * k:128 * (k + 1), 128 * h:128 * (h + 1)]

    aggb_h = np.stack([agg_b[:128], agg_b[128:]], axis=1)  # [128, 2]

    # selector B [64, 512] fp16
    b_h = _mh_row_meta(BIT_CVT_BIAS)[0].astype(np.float16)

    cmp_h = MH_CEQ.reshape(4, 128).T.astype(np.float32).copy()   # [128, 4]

    return {
        "tg": np.ascontiguousarray(tg.astype(BF16)),
        "wp": np.ascontiguousarray(wp_h.astype(BF16)),
        "mlpw": np.ascontiguousarray(mlpw_h.astype(BF16)),
        "mlpb": np.ascontiguousarray(mlp_b.astype(BF16).reshape(1, 256)),
        "aggb": np.ascontiguousarray(aggb_h),
        "cmpc": cmp_h,
        "bsel": np.ascontiguousarray(b_h),
        "ident": np.eye(128, dtype=np.float32).astype(BF16),
        "gbase": np.ascontiguousarray(
            np.repeat(np.asarray(G_BASES, np.int16)[None, :, None], 32, axis=2)
            .reshape(1, G * 32).repeat(128, axis=0)),  # [128, 224]
    }


def _pack_entity(ent):
    """Per-core entity-derived arrays: entT fp16 [64, E_CORE], gidx int16."""
    e_core = ent.shape[0]
    ntiles = e_core // TILE_E
    f = np.zeros((e_core, FT_ROWS), np.float16)
    f[:, FT_SP] = ent[:, SPECIES]
    f[:, FT_AB] = ent[:, ABILITY]
    f[:, FT_IT] = ent[:, ITEM]
    for i, feat in enumerate(SCALAR_FEATS):
        f[:, FT_SC0 + i] = ent[:, feat]
    for b, feat in enumerate(BOOST_FEATS):
        f[:, FT_BOOST0 + b] = ent[:, feat]
    words = ent[:, VOL0:TC1 + 1]            # 11 words
    for wi in range(N_WORDS):
        f[:, FT_BYTE0 + 2 * wi] = words[:, wi] & 0xFF
        f[:, FT_BYTE0 + 2 * wi + 1] = words[:, wi] >> 8
    for m in range(4):
        f[:, FT_MOVE0 + m] = ent[:, MOVE0 + m]
    f[:, FT_CONST1] = 1.0
    ent_t = np.ascontiguousarray(f.T)       # [64, e_core]

    v = ent[:, [SPECIES, ABILITY, ITEM, MOVE0, MOVE0 + 1, MOVE0 + 2, MOVE0 + 3]]
    v = v.astype(np.int16).reshape(ntiles, 32, 16, G)    # [t, s, p, g]
    gidx16 = v.transpose(2, 0, 3, 1).reshape(16, ntiles * G * 32)
    # dma_gather ucode: each of the 8 Q7 cores reads its own 16-partition
    # group, so the index block is replicated 8x along partitions.
    gidx = np.ascontiguousarray(np.tile(gidx16, (8, 1)))
    return ent_t, gidx


# ---------------------------------------------------------------- bass build
@functools.lru_cache(maxsize=4)
def _build(e_core):
    ntiles = e_core // TILE_E
    dt = mybir.dt
    nc = bacc.Bacc("TRN2", target_bir_lowering=False, debug=False)

    d_entT = nc.dram_tensor("entT", [FT_ROWS, e_core], dt.float16, kind="ExternalInput").ap()
    d_gidx = nc.dram_tensor("gidx", [128, ntiles * G * 32], dt.int16, kind="ExternalInput").ap()
    d_tg = nc.dram_tensor("tg", [TG_ROWS, 256], dt.bfloat16, kind="ExternalInput").ap()
    d_wp = nc.dram_tensor("wp", [128, 1024], dt.bfloat16, kind="ExternalInput").ap()
    d_mlpw = nc.dram_tensor("mlpw", [128, 512], dt.bfloat16, kind="ExternalInput").ap()
    d_mlpb = nc.dram_tensor("mlpb", [1, 256], dt.bfloat16, kind="ExternalInput").ap()
    d_aggb = nc.dram_tensor("aggb", [128, 2], dt.float32, kind="ExternalInput").ap()
    d_cmpc = nc.dram_tensor("cmpc", [128, 4], dt.float32, kind="ExternalInput").ap()
    d_bsel = nc.dram_tensor("bsel", [FT_ROWS, MH_ROWS], dt.float16, kind="ExternalInput").ap()
    d_ident = nc.dram_tensor("ident", [128, 128], dt.bfloat16, kind="ExternalInput").ap()
    d_gbase = nc.dram_tensor("gbase", [128, G * 32], dt.int16, kind="ExternalInput").ap()
    d_outT = nc.dram_tensor("outT", [256, e_core], dt.float32, kind="ExternalOutput").ap()

    with tile.TileContext(nc) as tc, ExitStack() as ctx:
        cpool = ctx.enter_context(tc.tile_pool(name="consts", bufs=1))
        wpool = ctx.enter_context(tc.tile_pool(name="work", bufs=3))
        gpool = ctx.enter_context(tc.tile_pool(name="gather", bufs=3))
        ppool = ctx.enter_context(tc.tile_pool(name="psum", bufs=1, space="PSUM"))

        # ---- persistent constants
        entT = cpool.tile([FT_ROWS, e_core], dt.float16, tag="entT")
        nc.sync.dma_start(entT[:], d_entT)
        gidx = cpool.tile([128, ntiles * G * 32], dt.int16, tag="gidx")
        nc.sync.dma_start(gidx[:], d_gidx)
        wp = cpool.tile([128, 1024], dt.bfloat16, tag="wp")
        nc.sync.dma_start(wp[:], d_wp)
        mlpw = cpool.tile([128, 512], dt.bfloat16, tag="mlpw")
        nc.sync.dma_start(mlpw[:], d_mlpw)
        mlpb = cpool.tile([1, 256], dt.bfloat16, tag="mlpb")
        nc.sync.dma_start(mlpb[:], d_mlpb)
        aggb = cpool.tile([128, 2], dt.float32, tag="aggb")
        nc.sync.dma_start(aggb[:], d_aggb)
        cmpc = cpool.tile([128, 4], dt.float32, tag="cmpc")
        nc.sync.dma_start(cmpc[:], d_cmpc)
        bsel = cpool.tile([FT_ROWS, MH_ROWS], dt.float16, tag="bsel")
        nc.sync.dma_start(bsel[:], d_bsel)
        ident = cpool.tile([128, 128], dt.bfloat16, tag="ident")
        nc.sync.dma_start(ident[:], d_ident)
        gbase = cpool.tile([128, G * 32], dt.int16, tag="gbase")
        nc.sync.dma_start(gbase[:], d_gbase)

        # persistent gather-index buffer (indices replicated per 16-row group)
        idxb = cpool.tile([128, ntiles * G * 32], dt.int16, tag="idxb")

        # all gather indices up-front so gathers chain without DVE deps
        for t in range(ntiles):
            isl = slice(t * G * 32, (t + 1) * G * 32)
            nc.vector.tensor_tensor(
                idxb[:, isl], gidx[:, isl], gbase[:], mybir.AluOpType.add)

        for t in range(ntiles):
            es = slice(t * TILE_E, (t + 1) * TILE_E)
            isl = slice(t * G * 32, (t + 1) * G * 32)

            # 7*TILE_E row gather from Tg (HBM), transposed output
            gpl = gpool.tile([128, 2 * G * TILE_E], dt.bfloat16, tag="gpl")
            gpl3 = gpl[:].rearrange("p (c j) -> p c j", c=2)
            nc.gpsimd.dma_gather(
                out_ap=gpl3,
                in_ap=d_tg,
                idxs_ap=idxb[:, isl],
                num_idxs=G * TILE_E,
                num_idxs_reg=G * TILE_E,
                elem_size=256,
                transpose=True,
                single_packet=False,
            )

            # selector matmuls: raw[c] = B_c.T @ featT
            raws = []
            for c in range(4):
                raw = ppool.tile([128, TILE_E], dt.float32, tag=f"raw{c}")
                nc.tensor.matmul(
                    raw[:], bsel[:, c * 128:(c + 1) * 128], entT[:, es],
                    start=True, stop=True)
                raws.append(raw)

            # multi-hot construction
            mh = wpool.tile([128, 4 * TILE_E], dt.bfloat16, tag="mh")
            cvti = wpool.tile([128, TILE_E], dt.int16, tag="cvti")
            cvt2 = wpool.tile([128, TILE_E], dt.int16, tag="cvt2")
            rawh = wpool.tile([128, TILE_E], dt.float32, tag="rawh")
            for (c, lo, hi, kind) in MH_OPS:
                dst = mh[lo:hi, c * TILE_E:(c + 1) * TILE_E]
                src = raws[c][lo:hi, :]
                if kind == "eq":
                    nc.vector.tensor_scalar(
                        dst, src, cmpc[lo:hi, c:c + 1], None,
                        mybir.AluOpType.is_equal)
                elif kind == "bit":
                    # raw = v*2^-jj + bias; bit = (v>>jj) - 2*(v>>(jj+1)),
                    # integer shifts realized as RNE-safe f32->i16 casts
                    # (int16 bitwise ops are ~8x slower than casts on DVE).
                    nc.vector.tensor_scalar(
                        rawh[lo:hi, :], src, 0.5, BIT_CVT_BIAS * 0.5,
                        mybir.AluOpType.mult, mybir.AluOpType.add)
                    nc.vector.tensor_copy(cvti[lo:hi, :], src)
                    nc.vector.tensor_copy(cvt2[lo:hi, :], rawh[lo:hi, :])
                    nc.vector.scalar_tensor_tensor(
                        dst, cvt2[lo:hi, :], -2.0, cvti[lo:hi, :],
                        mybir.AluOpType.mult, mybir.AluOpType.add)
                elif kind == "ge":
                    nc.vector.tensor_scalar(
                        dst, src, cmpc[lo:hi, c:c + 1], None,
                        mybir.AluOpType.is_ge)

            # gather-plane sum (+ agg_b on the final combine)
            def plane(g):
                return gpl3[:, :, g * TILE_E:(g + 1) * TILE_E]

            a0 = wpool.tile([128, 2 * TILE_E], dt.bfloat16, tag="a0")
            a03 = a0[:].rearrange("p (c j) -> p c j", c=2)
            nc.vector.tensor_tensor(a03, plane(0), plane(1), mybir.AluOpType.add)
            a1 = wpool.tile([128, 2 * TILE_E], dt.bfloat16, tag="a1")
            a13 = a1[:].rearrange("p (c j) -> p c j", c=2)
            nc.vector.tensor_tensor(a13, plane(2), plane(3), mybir.AluOpType.add)
            a2 = wpool.tile([128, 2 * TILE_E], dt.bfloat16, tag="a2")
            a23 = a2[:].rearrange("p (c j) -> p c j", c=2)
            nc.vector.tensor_tensor(a23, plane(4), plane(5), mybir.AluOpType.add)
            nc.vector.tensor_tensor(a03, a03, a13, mybir.AluOpType.add)
            nc.vector.tensor_tensor(a23, a23, plane(6), mybir.AluOpType.add)
            gs = wpool.tile([128, 2 * TILE_E], dt.bfloat16, tag="gs")
            for h in range(2):
                nc.vector.scalar_tensor_tensor(
                    gs[:, h * TILE_E:(h + 1) * TILE_E],
                    a0[:, h * TILE_E:(h + 1) * TILE_E],
                    aggb[:, h:h + 1],
                    a2[:, h * TILE_E:(h + 1) * TILE_E],
                    mybir.AluOpType.add, mybir.AluOpType.add)

            # x1 = gathers + one-hot part (PSUM accumulation)
            x1 = []
            for h in range(2):
                p = ppool.tile([128, TILE_E], dt.float32, tag=f"x1_{h}")
                nc.tensor.matmul(
                    p[:], ident[:], gs[:, h * TILE_E:(h + 1) * TILE_E],
                    start=True, stop=False)
                for c in range(4):
                    nc.tensor.matmul(
                        p[:], wp[:, (c * 2 + h) * 128:(c * 2 + h + 1) * 128],
                        mh[:, c * TILE_E:(c + 1) * TILE_E],
                        start=False, stop=(c == 3))
                x1.append(p)

            # relu -> xr (bf16)
            xr = wpool.tile([128, 2 * TILE_E], dt.bfloat16, tag="xr")
            for h in range(2):
                nc.scalar.activation(
                    xr[:, h * TILE_E:(h + 1) * TILE_E], x1[h][:],
                    mybir.ActivationFunctionType.Relu)

            # out = xr @ mlp_w + mask*mlp_b
            mrow = mh[MH_MASK:MH_MASK + 1, 0:TILE_E]    # (sp>=2) row, chunk 0
            for h in range(2):
                po = ppool.tile([128, TILE_E], dt.float32, tag=f"out_{h}")
                for k in range(2):
                    nc.tensor.matmul(
                        po[:], mlpw[:, (k * 2 + h) * 128:(k * 2 + h + 1) * 128],
                        xr[:, k * TILE_E:(k + 1) * TILE_E],
                        start=(k == 0), stop=False)
                nc.tensor.matmul(
                    po[:], mlpb[:, h * 128:(h + 1) * 128], mrow,
                    start=False, stop=True)
                ob = wpool.tile([128, TILE_E], dt.float32, tag=f"ob{h}")
                nc.scalar.activation(
                    ob[:], po[:], mybir.ActivationFunctionType.Copy)
                nc.sync.dma_start(d_outT[h * 128:(h + 1) * 128, es], ob[:])

    nc.compile()
    return nc


# ---------------------------------------------------------------- entry
def _make_in_maps(inputs, n_cores, e_core):
    ent = np.asarray(inputs["entity"], np.int32)
    w = _pack_weights(inputs)
    in_maps = []
    for i in range(n_cores):
        ent_t, gidx = _pack_entity(ent[i * e_core:(i + 1) * e_core])
        in_maps.append({
            "entT": ent_t, "gidx": gidx, "tg": w["tg"], "wp": w["wp"],
            "mlpw": w["mlpw"], "mlpb": w["mlpb"], "aggb": w["aggb"],
            "cmpc": w["cmpc"],
            "bsel": w["bsel"], "ident": w["ident"], "gbase": w["gbase"],
        })
    return in_maps


def kernel(**inputs):
    nc = _build(E_CORE)
    in_maps = _make_in_maps(inputs, N_CORES, E_CORE)
    res = run_bass_kernel_spmd(nc, in_maps, list(range(N_CORES)))
    out = np.concatenate(
        [np.ascontiguousarray(res.results[i]["outT"].T) for i in range(N_CORES)],
        axis=0)
    return out


def run_traced(inputs):
    """test.py helper: returns (output, exec_time_ns)."""
    nc = _build(E_CORE)
    in_maps = _make_in_maps(inputs, N_CORES, E_CORE)
    # warmup: connects the axon client (profile hook needs it) + NEFF cache
    run_bass_kernel_spmd(nc, in_maps, list(range(N_CORES)))
    res = run_bass_kernel_spmd(nc, in_maps, list(range(N_CORES)), trace=True)
    out = np.concatenate(
        [np.ascontiguousarray(res.results[i]["outT"].T) for i in range(N_CORES)],
        axis=0)
    return out, res.exec_time_ns
